# revision 1
# baseline (speedup 1.0000x reference)
import numpy as np

B, T, C = 4, 2048, 1024
H_PER_CORE = 8
HL = 512
D = 64
QC = 512
NQC = T // QC
NKC = T // 128
N_CORES = 8

_CACHE = {}


def _emit(nc, tc, tile, mybir, io):
    import concourse.bass as bass
    f32, f32r = mybir.dt.float32, mybir.dt.float32r
    Exp = mybir.ActivationFunctionType.Exp
    xT, wq, wk, wv, wc, maskw, ones, out = (
        io["xT"], io["wq"], io["wk"], io["wv"], io["wc"],
        io["maskw"], io["ones"], io["out"],
    )

    from contextlib import ExitStack

    with ExitStack() as ctx:
        persist = ctx.enter_context(tc.tile_pool(name="persist", bufs=1))
        qt = persist.tile([128, 4, T], f32r)
        kt = persist.tile([128, 4, T], f32r)
        vp = persist.tile([128, NKC, H_PER_CORE * (D + 1)], f32r)
        ones_view = vp.rearrange("p mt (h c) -> p mt h c", c=D + 1)[:, :, :, D]
        nc.sync.dma_start(out=ones_view, in_=ones.bitcast(f32r))

        def load_xt(pool, tag, n):
            xts = []
            for kc2 in range(4):
                t = pool.tile([128, 2, QC], f32r, tag=tag)
                for j in range(2):
                    kc = kc2 * 2 + j
                    nc.sync.dma_start(
                        out=t[:, j, :],
                        in_=xT.bitcast(f32r)[kc * 128:(kc + 1) * 128,
                                             n * QC:(n + 1) * QC])
                xts.append(t)
            return lambda kc: xts[kc // 2][:, kc % 2, :]

        with (
            tc.tile_pool(name="wqk", bufs=1) as wpool,
            tc.tile_pool(name="xtp", bufs=6) as xtp,
            tc.tile_pool(name="ps1", bufs=8, space="PSUM") as ps1,
        ):
            wq_sb = wpool.tile([128, 8, HL], f32r, tag="wq")
            wk_sb = wpool.tile([128, 8, HL], f32r, tag="wk")
            wv_sb = wpool.tile([128, 8, HL], f32r, tag="wv")
            for kc in range(8):
                nc.gpsimd.dma_start(
                    out=wq_sb[:, kc, :],
                    in_=wq.bitcast(f32r)[kc * 128:(kc + 1) * 128, :])
                nc.gpsimd.dma_start(
                    out=wk_sb[:, kc, :],
                    in_=wk.bitcast(f32r)[kc * 128:(kc + 1) * 128, :])
                nc.gpsimd.dma_start(
                    out=wv_sb[:, kc, :],
                    in_=wv.bitcast(f32r)[kc * 128:(kc + 1) * 128, :])
            for n in range(NQC):
                xs = load_xt(xtp, "xt", n)
                for mc in range(4):
                    pq = ps1.tile([128, QC], f32, tag="p1")
                    for kc in range(8):
                        nc.tensor.matmul(
                            out=pq[:], lhsT=wq_sb[:, kc, mc * 128:(mc + 1) * 128],
                            rhs=xs(kc), start=(kc == 0), stop=(kc == 7))
                    nc.scalar.copy(qt[:, mc, n * QC:(n + 1) * QC], pq[:])
                    pk = ps1.tile([128, QC], f32, tag="p1")
                    for kc in range(8):
                        nc.tensor.matmul(
                            out=pk[:], lhsT=wk_sb[:, kc, mc * 128:(mc + 1) * 128],
                            rhs=xs(kc), start=(kc == 0), stop=(kc == 7))
                    nc.vector.tensor_copy(kt[:, mc, n * QC:(n + 1) * QC], pk[:])
                for mt in range(4):
                    gm = n * 4 + mt
                    pv = ps1.tile([128, HL], f32, tag="p1")
                    for kc in range(8):
                        nc.tensor.matmul(
                            out=pv[:], lhsT=xs(kc)[:, mt * 128:(mt + 1) * 128],
                            rhs=wv_sb[:, kc, :], start=(kc == 0), stop=(kc == 7))
                    for h in range(H_PER_CORE):
                        nc.vector.tensor_copy(
                            vp[:, gm, h * (D + 1):h * (D + 1) + D],
                            pv[:, h * D:(h + 1) * D])

        with tc.tile_pool(name="otp", bufs=1) as otpool, \
             tc.tile_pool(name="wcp", bufs=1) as wcpool:
            ot = otpool.tile([128, 4, T], f32r)
            wc_sb = wcpool.tile([128, 4, C], f32r)
            nc.sync.dma_start(
                out=wc_sb, in_=wc.bitcast(f32r).rearrange("(kd p) m -> p kd m", p=128))

            with (
                tc.tile_pool(name="mk", bufs=1) as mkpool,
                tc.tile_pool(name="etp", bufs=4) as etp,
                tc.tile_pool(name="smp", bufs=4) as smp,
                tc.tile_pool(name="drp", bufs=8, space="DRAM") as drp,
                tc.tile_pool(name="psw", bufs=2, space="PSUM") as psw,
                tc.tile_pool(name="pso", bufs=2, space="PSUM") as pso,
            ):
                mask_sb = mkpool.tile([128, 4 * QC], f32r)
                nc.sync.dma_start(out=mask_sb, in_=maskw.bitcast(f32r))

                def emit_batch(h, qc, po, bi, cnt):
                    r0 = (h % 2) * 64
                    chh = h // 2
                    K = 4 * qc + 4
                    pw = psw.tile([128, 3, QC], f32, tag="pw")
                    for i in range(cnt):
                        kc = bi * 3 + i
                        nc.tensor.matmul(
                            out=pw[:, i, :],
                            lhsT=kt[r0:r0 + 64, chh, kc * 128:(kc + 1) * 128],
                            rhs=qt[r0:r0 + 64, chh, qc * QC:(qc + 1) * QC],
                            start=True, stop=True, tile_position=(r0, 0))
                    ew = etp.tile([128, 3, QC], f32r, tag="et")
                    nc.scalar.activation(
                        ew[:, 0:cnt, :], pw[:, 0:cnt, :], Exp, scale=0.125)
                    for i in range(cnt):
                        d = bi * 3 + i - 4 * qc
                        if d >= 0:
                            nc.vector.tensor_mul(
                                ew[:, i, :], ew[:, i, :],
                                mask_sb[:, d * QC:(d + 1) * QC])
                    for i in range(cnt):
                        kc = bi * 3 + i
                        nc.tensor.matmul(
                            out=po[0:D + 1, :],
                            lhsT=vp[:, kc, h * (D + 1):(h + 1) * (D + 1)],
                            rhs=ew[:, i, :],
                            start=(kc == 0), stop=(kc == K - 1))

                def evict(h, qc, po):
                    r0 = (h % 2) * 64
                    chh = h // 2
                    ot_slice = ot[r0:r0 + 64, chh, qc * QC:(qc + 1) * QC]
                    nc.vector.tensor_copy(ot_slice, po[0:64, :])
                    d1 = smp.tile([1, QC], f32, tag="d1")
                    nc.vector.tensor_copy(d1[:], po[D:D + 1, :])
                    scr = drp.tile([1, QC], f32, tag="scr")
                    nc.sync.dma_start(out=scr[:], in_=d1[:])
                    db = smp.tile([128, QC], f32, tag="db")
                    s0 = scr[:]
                    nc.gpsimd.dma_start(
                        out=db[:],
                        in_=bass.AP(tensor=s0.tensor, offset=s0.offset,
                                    ap=[[0, 128], [1, QC]]))
                    nc.vector.reciprocal_approx_fast(db[:], db[:])
                    nc.vector.tensor_mul(ot_slice, ot_slice, db[r0:r0 + 64, :])

                for hp in range(H_PER_CORE // 2):
                    ha, hb = 2 * hp, 2 * hp + 1
                    for qc in range(NQC):
                        K = 4 * qc + 4
                        po_a = pso.tile([128, QC], f32, tag="po")
                        po_b = pso.tile([128, QC], f32, tag="po")
                        nbatches = (K + 2) // 3
                        for bi in range(nbatches):
                            cnt = min(3, K - bi * 3)
                            emit_batch(ha, qc, po_a, bi, cnt)
                            emit_batch(hb, qc, po_b, bi, cnt)
                        evict(ha, qc, po_a)
                        evict(hb, qc, po_b)

            with (
                tc.tile_pool(name="stp", bufs=4) as stp,
                tc.tile_pool(name="ps3", bufs=6, space="PSUM") as ps3,
            ):
                for mt in range(NKC):
                    for n2 in range(2):
                        pc = ps3.tile([128, QC], f32, tag="pc")
                        for kd in range(4):
                            nc.tensor.matmul(
                                out=pc[:],
                                lhsT=ot[:, kd, mt * 128:(mt + 1) * 128],
                                rhs=wc_sb[:, kd, n2 * QC:(n2 + 1) * QC],
                                start=(kd == 0), stop=(kd == 3))
                        st = stp.tile([128, QC], f32, tag="st")
                        if (mt + n2) % 2 == 0:
                            nc.vector.tensor_copy(st[:], pc[:])
                        else:
                            nc.scalar.copy(st[:], pc[:])
                        nc.sync.dma_start(
                            out=out[mt * 128:(mt + 1) * 128,
                                    n2 * QC:(n2 + 1) * QC],
                            in_=st[:])


def build_program():
    if "nc" in _CACHE:
        return _CACHE["nc"]
    import concourse.bacc as bacc
    import concourse.tile as tile
    from concourse import mybir

    f32 = mybir.dt.float32
    nc = bacc.Bacc("TRN2", target_bir_lowering=False, debug=False,
                   num_devices=N_CORES)
    io = {
        "xT": nc.dram_tensor("xT", [C, T], f32, kind="ExternalInput").ap(),
        "wq": nc.dram_tensor("wq", [C, HL], f32, kind="ExternalInput").ap(),
        "wk": nc.dram_tensor("wk", [C, HL], f32, kind="ExternalInput").ap(),
        "wv": nc.dram_tensor("wv", [C, HL], f32, kind="ExternalInput").ap(),
        "wc": nc.dram_tensor("wc", [HL, C], f32, kind="ExternalInput").ap(),
        "maskw": nc.dram_tensor("maskw", [128, 4 * QC], f32,
                                kind="ExternalInput").ap(),
        "ones": nc.dram_tensor("ones", [128, NKC, H_PER_CORE], f32,
                               kind="ExternalInput").ap(),
        "out": nc.dram_tensor("out", [T, C], f32, kind="ExternalOutput").ap(),
    }
    with tile.TileContext(nc) as tc:
        _emit(nc, tc, tile, mybir, io)
    nc.compile()
    _CACHE["nc"] = nc
    return nc


def make_in_maps(x, Wq, Wk, Wv, Wc):
    x = np.asarray(x, dtype=np.float32)
    Wq = np.asarray(Wq, dtype=np.float32)
    Wk = np.asarray(Wk, dtype=np.float32)
    Wv = np.asarray(Wv, dtype=np.float32)
    Wc = np.asarray(Wc, dtype=np.float32)

    i_idx = np.arange(128)[:, None]
    j_idx = np.arange(QC)[None, :]
    maskw = np.concatenate(
        [(j_idx >= i_idx + 128 * d) for d in range(4)], axis=1
    ).astype(np.float32)
    ones = np.ones((128, NKC, H_PER_CORE), dtype=np.float32)

    in_maps = []
    for b in range(B):
        xT = np.ascontiguousarray(x[b].T)
        for g in range(2):
            sl = slice(g * HL, (g + 1) * HL)
            in_maps.append({
                "xT": xT,
                "wq": np.ascontiguousarray(Wq[:, sl]),
                "wk": np.ascontiguousarray(Wk[:, sl]),
                "wv": np.ascontiguousarray(Wv[:, sl]),
                "wc": np.ascontiguousarray(Wc[sl, :]),
                "maskw": maskw,
                "ones": ones,
            })
    return in_maps


def kernel(x, Wq, Wk, Wv, Wc, bc):
    from concourse.bass_utils import run_bass_kernel_spmd

    nc = build_program()
    in_maps = make_in_maps(x, Wq, Wk, Wv, Wc)
    res = run_bass_kernel_spmd(nc, in_maps, core_ids=list(range(N_CORES)))
    bc = np.asarray(bc, dtype=np.float32)
    out = np.empty((B, T, C), dtype=np.float32)
    for b in range(B):
        out[b] = res.results[2 * b]["out"] + res.results[2 * b + 1]["out"] + bc
    return out



# revision 10
# speedup vs baseline: 1.0301x; 1.0301x over previous
import numpy as np
import ml_dtypes

B, T, C = 4, 2048, 1024
H_PER_CORE = 8
HL = 512
D = 64
QC = 512
NQC = T // QC
NKC = T // 128
N_CORES = 8

_CACHE = {}


def _emit(nc, tc, tile, mybir, io):
    import concourse.bass as bass
    f32, bf16 = mybir.dt.float32, mybir.dt.bfloat16
    Exp = mybir.ActivationFunctionType.Exp
    xT, wq, wk, wv, wc, tril, out = (
        io["xT"], io["wq"], io["wk"], io["wv"], io["wc"],
        io["tril"], io["out"],
    )

    from contextlib import ExitStack

    with ExitStack() as ctx:
        persist = ctx.enter_context(tc.tile_pool(name="persist", bufs=1))
        qt = persist.tile([128, 4, T], bf16)
        kt = persist.tile([128, 4, T], bf16)
        ot = persist.tile([128, 4, T], bf16)
        vp = persist.tile([128, NKC, H_PER_CORE, D + 1], bf16)
        wq_sb = persist.tile([128, 8, HL], bf16, tag="wqs")
        wk_sb = persist.tile([128, 8, HL], bf16, tag="wks")
        wv_sb = persist.tile([128, 8, HL], bf16, tag="wvs")
        wc_sb = persist.tile([128, 4, C], bf16, tag="wcs")
        tril_sb = persist.tile([128, 2, 128], bf16, tag="tril")

        nc.gpsimd.dma_start(
            out=wq_sb, in_=wq.rearrange("(kc p) m -> p kc m", p=128))
        nc.gpsimd.dma_start(
            out=wk_sb, in_=wk.rearrange("(kc p) m -> p kc m", p=128))
        nc.gpsimd.dma_start(
            out=wv_sb, in_=wv.rearrange("(kc p) m -> p kc m", p=128))
        nc.gpsimd.dma_start(
            out=wc_sb, in_=wc.rearrange("(kd p) m -> p kd m", p=128))
        nc.vector.memset(vp[:, :, :, D], 1.0)
        nc.sync.dma_start(out=tril_sb[:, 0, :], in_=tril)
        nc.sync.dma_start(out=tril_sb[:, 1, :], in_=tril)

        pA = ctx.enter_context(tc.tile_pool(name="pA", bufs=2, space="PSUM"))
        pwp = ctx.enter_context(tc.tile_pool(name="pw", bufs=3, space="PSUM"))
        pop = ctx.enter_context(tc.tile_pool(name="po", bufs=3, space="PSUM"))
        xtp = ctx.enter_context(tc.tile_pool(name="xtp", bufs=8))
        ewp = ctx.enter_context(tc.tile_pool(name="ewp", bufs=4))
        dbp = ctx.enter_context(tc.tile_pool(name="dbp", bufs=2))
        stp = ctx.enter_context(tc.tile_pool(name="stp", bufs=4))
        drp = ctx.enter_context(tc.tile_pool(name="drp", bufs=4, space="DRAM"))

        x_tiles = {}

        def emit_x_dma(n):
            ts = []
            for kc2 in range(4):
                t = xtp.tile([128, 2, QC], bf16, tag="xt")
                for j in range(2):
                    kc = kc2 * 2 + j
                    nc.sync.dma_start(
                        out=t[:, j, :],
                        in_=xT[kc * 128:(kc + 1) * 128,
                               n * QC:(n + 1) * QC])
                ts.append(t)
            x_tiles[n] = ts

        def xs(n, kc):
            return x_tiles[n][kc // 2][:, kc % 2, :]

        def qk_group(n, mc, wsb, dst):
            p = pA.tile([128, QC], f32, tag="pA")
            for kc in range(8):
                nc.tensor.matmul(
                    out=p[:], lhsT=wsb[:, kc, mc * 128:(mc + 1) * 128],
                    rhs=xs(n, kc), start=(kc == 0), stop=(kc == 7))
            nc.vector.tensor_copy(dst[:, mc, n * QC:(n + 1) * QC], p[:])

        def v_group(n, mt):
            p = pA.tile([128, QC], f32, tag="pA")
            for kc in range(8):
                nc.tensor.matmul(
                    out=p[:], lhsT=xs(n, kc)[:, mt * 128:(mt + 1) * 128],
                    rhs=wv_sb[:, kc, :], start=(kc == 0), stop=(kc == 7))
            gm = n * 4 + mt
            nc.vector.tensor_copy(
                vp[:, gm, :, 0:D],
                p.rearrange("p (h d) -> p h d", d=D))

        def p3_group(qc, j, n2):
            mt = 4 * qc + j
            p = pA.tile([128, QC], f32, tag="pA")
            for kd in range(4):
                nc.tensor.matmul(
                    out=p[:],
                    lhsT=ot[:, kd, mt * 128:(mt + 1) * 128],
                    rhs=wc_sb[:, kd, n2 * QC:(n2 + 1) * QC],
                    start=(kd == 0), stop=(kd == 3))
            st = stp.tile([128, QC], bf16, tag="st")
            nc.vector.tensor_copy(st[:], p[:])
            nc.sync.dma_start(
                out=out[mt * 128:(mt + 1) * 128, n2 * QC:(n2 + 1) * QC],
                in_=st[:])

        def so_block(qc):
            K = 4 * qc + 4
            for hp in range(4):
                ha, hb = 2 * hp, 2 * hp + 1
                po_a = pop.tile([128, QC], f32, tag="po")
                po_b = pop.tile([128, QC], f32, tag="po")
                for kc in range(K):
                    off = (kc - 4 * qc) * 128 if kc >= 4 * qc else 0
                    pw_a = pwp.tile([128, QC], f32, tag="pw")
                    pw_b = pwp.tile([128, QC], f32, tag="pw")
                    for r0, pw_t in ((0, pw_a), (64, pw_b)):
                        nc.tensor.matmul(
                            out=pw_t[:, off:QC],
                            lhsT=kt[r0:r0 + 64, hp,
                                    kc * 128:(kc + 1) * 128],
                            rhs=qt[r0:r0 + 64, hp,
                                   qc * QC + off:(qc + 1) * QC],
                            start=True, stop=True, tile_position=(r0, 0))
                    ew = ewp.tile([128, 2, QC], bf16, tag="ew")
                    for hi, pw_t in ((0, pw_a), (1, pw_b)):
                        nc.scalar.activation(
                            ew[:, hi, off:QC], pw_t[:, off:QC], Exp,
                            scale=0.125)
                    if kc >= 4 * qc:
                        nc.gpsimd.tensor_mul(
                            ew[:, :, off:off + 128],
                            ew[:, :, off:off + 128],
                            tril_sb[:, :, :])
                    for hi, (h, po_t) in ((0, (ha, po_a)),
                                          (1, (hb, po_b))):
                        nc.tensor.matmul(
                            out=po_t[0:D + 1, off:QC],
                            lhsT=vp[:, kc, h, :],
                            rhs=ew[:, hi, off:QC],
                            start=(kc == 0), stop=(kc == K - 1),
                            skip_group_check=True)
                    yield
                qsl = slice(qc * QC, (qc + 1) * QC)
                nc.vector.tensor_copy(ot[0:64, hp, qsl], po_a[0:D, :])
                nc.vector.tensor_copy(ot[64:128, hp, qsl], po_b[0:D, :])
                d_sb = dbp.tile([1, 2, QC], f32, tag="dsb")
                nc.vector.tensor_copy(d_sb[0:1, 0, :], po_a[D:D + 1, :])
                nc.vector.tensor_copy(d_sb[0:1, 1, :], po_b[D:D + 1, :])
                dr = drp.tile([2, QC], f32, tag="dr")
                nc.sync.dma_start(out=dr[:], in_=d_sb[:])
                db = dbp.tile([128, QC], f32, tag="db")
                d0 = dr[:]
                nc.gpsimd.dma_start(
                    out=db[:],
                    in_=bass.AP(tensor=d0.tensor, offset=d0.offset,
                                ap=[[QC, 2], [0, 64], [1, QC]]))
                nc.vector.reciprocal_approx_fast(db[:], db[:])
                nc.vector.tensor_mul(ot[:, hp, qsl], ot[:, hp, qsl], db[:])

        def block_fillers(n):
            fs = []
            if n + 1 < NQC:
                fs.append(lambda n=n: emit_x_dma(n + 1))
            if n < NQC:
                for mc in range(4):
                    fs.append(lambda n=n, mc=mc: qk_group(n, mc, wq_sb, qt))
                    fs.append(lambda n=n, mc=mc: qk_group(n, mc, wk_sb, kt))
                for mt in range(4):
                    fs.append(lambda n=n, mt=mt: v_group(n, mt))
            if 2 <= n:
                qc = n - 2
                for j in range(4):
                    for n2 in range(2):
                        fs.append(
                            lambda qc=qc, j=j, n2=n2: p3_group(qc, j, n2))
            return fs

        emit_x_dma(0)
        for n in range(6):
            fillers = block_fillers(n)
            if n == 0 or n == 5:
                for f in fillers:
                    f()
                continue
            qc = n - 1
            n_bi = 4 * (4 * qc + 4)
            rate = len(fillers) / n_bi
            acc = 0.0
            for _ in so_block(qc):
                acc += rate
                while acc >= 1.0 and fillers:
                    fillers.pop(0)()
                    acc -= 1.0
            for f in fillers:
                f()


def build_program():
    if "nc" in _CACHE:
        return _CACHE["nc"]
    import concourse.bacc as bacc
    import concourse.tile as tile
    from concourse import mybir

    f32, bf16 = mybir.dt.float32, mybir.dt.bfloat16
    nc = bacc.Bacc("TRN2", target_bir_lowering=False, debug=False,
                   num_devices=N_CORES)
    io = {
        "xT": nc.dram_tensor("xT", [C, T], bf16, kind="ExternalInput").ap(),
        "wq": nc.dram_tensor("wq", [C, HL], bf16, kind="ExternalInput").ap(),
        "wk": nc.dram_tensor("wk", [C, HL], bf16, kind="ExternalInput").ap(),
        "wv": nc.dram_tensor("wv", [C, HL], bf16, kind="ExternalInput").ap(),
        "wc": nc.dram_tensor("wc", [HL, C], bf16, kind="ExternalInput").ap(),
        "tril": nc.dram_tensor("tril", [128, 128], bf16,
                               kind="ExternalInput").ap(),
        "out": nc.dram_tensor("out", [T, C], bf16, kind="ExternalOutput").ap(),
    }
    with tile.TileContext(nc) as tc:
        _emit(nc, tc, tile, mybir, io)
    nc.compile()
    _CACHE["nc"] = nc
    return nc


def make_in_maps(x, Wq, Wk, Wv, Wc):
    bf16 = ml_dtypes.bfloat16
    x = np.asarray(x, dtype=np.float32)
    Wq = np.asarray(Wq, dtype=np.float32).astype(bf16)
    Wk = np.asarray(Wk, dtype=np.float32).astype(bf16)
    Wv = np.asarray(Wv, dtype=np.float32).astype(bf16)
    Wc = np.asarray(Wc, dtype=np.float32).astype(bf16)

    i_idx = np.arange(128)[:, None]
    j_idx = np.arange(128)[None, :]
    tril = (j_idx >= i_idx).astype(bf16)

    in_maps = []
    for b in range(B):
        xT = np.ascontiguousarray(x[b].T).astype(bf16)
        for g in range(2):
            sl = slice(g * HL, (g + 1) * HL)
            in_maps.append({
                "xT": xT,
                "wq": np.ascontiguousarray(Wq[:, sl]),
                "wk": np.ascontiguousarray(Wk[:, sl]),
                "wv": np.ascontiguousarray(Wv[:, sl]),
                "wc": np.ascontiguousarray(Wc[sl, :]),
                "tril": tril,
            })
    return in_maps


def kernel(x, Wq, Wk, Wv, Wc, bc):
    from concourse.bass_utils import run_bass_kernel_spmd

    nc = build_program()
    in_maps = make_in_maps(x, Wq, Wk, Wv, Wc)
    res = run_bass_kernel_spmd(nc, in_maps, core_ids=list(range(N_CORES)))
    bc = np.asarray(bc, dtype=np.float32)
    out = np.empty((B, T, C), dtype=np.float32)
    for b in range(B):
        out[b] = (res.results[2 * b]["out"].astype(np.float32)
                  + res.results[2 * b + 1]["out"].astype(np.float32) + bc)
    return out


# revision 11
# speedup vs baseline: 1.4784x; 1.4353x over previous
import numpy as np
import ml_dtypes

B, T, C = 4, 2048, 1024
H_PER_CORE = 8
HL = 512
D = 64
QC = 512
NQC = T // QC
NKC = T // 128
N_CORES = 8

_CACHE = {}


def _emit(nc, tc, tile, mybir, io):
    import concourse.bass as bass
    f32, bf16 = mybir.dt.float32, mybir.dt.bfloat16
    Exp = mybir.ActivationFunctionType.Exp
    xT, wq, wk, wv, wc, tril, out = (
        io["xT"], io["wq"], io["wk"], io["wv"], io["wc"],
        io["tril"], io["out"],
    )

    from contextlib import ExitStack

    with ExitStack() as ctx:
        persist = ctx.enter_context(tc.tile_pool(name="persist", bufs=1))
        qt = persist.tile([128, 4, T], bf16)
        kt = persist.tile([128, 4, T], bf16)
        ot = persist.tile([128, 4, T], bf16)
        vp = persist.tile([128, NKC, H_PER_CORE, D + 1], bf16)
        wq_sb = persist.tile([128, 8, HL], bf16, tag="wqs")
        wk_sb = persist.tile([128, 8, HL], bf16, tag="wks")
        wv_sb = persist.tile([128, 8, HL], bf16, tag="wvs")
        wc_sb = persist.tile([128, 4, C], bf16, tag="wcs")
        tril_sb = persist.tile([128, 2, 128], bf16, tag="tril")

        nc.gpsimd.dma_start(
            out=wq_sb, in_=wq.rearrange("(kc p) m -> p kc m", p=128))
        nc.gpsimd.dma_start(
            out=wk_sb, in_=wk.rearrange("(kc p) m -> p kc m", p=128))
        nc.gpsimd.dma_start(
            out=wv_sb, in_=wv.rearrange("(kc p) m -> p kc m", p=128))
        nc.gpsimd.dma_start(
            out=wc_sb, in_=wc.rearrange("(kd p) m -> p kd m", p=128))
        nc.vector.memset(vp[:, :, :, D], 1.0)
        nc.sync.dma_start(out=tril_sb[:, 0, :], in_=tril)
        nc.sync.dma_start(out=tril_sb[:, 1, :], in_=tril)

        pA = ctx.enter_context(tc.tile_pool(name="pA", bufs=2, space="PSUM"))
        pwp = ctx.enter_context(tc.tile_pool(name="pw", bufs=2, space="PSUM"))
        pop = ctx.enter_context(tc.tile_pool(name="po", bufs=2, space="PSUM"))
        xtp = ctx.enter_context(tc.tile_pool(name="xtp", bufs=8))
        ewp = ctx.enter_context(tc.tile_pool(name="ewp", bufs=6))
        dbp = ctx.enter_context(tc.tile_pool(name="dbp", bufs=2))
        stp = ctx.enter_context(tc.tile_pool(name="stp", bufs=4))
        drp = ctx.enter_context(tc.tile_pool(name="drp", bufs=4, space="DRAM"))

        x_tiles = {}

        def emit_x_dma(n):
            ts = []
            for kc2 in range(4):
                t = xtp.tile([128, 2, QC], bf16, tag="xt")
                for j in range(2):
                    kc = kc2 * 2 + j
                    nc.sync.dma_start(
                        out=t[:, j, :],
                        in_=xT[kc * 128:(kc + 1) * 128,
                               n * QC:(n + 1) * QC])
                ts.append(t)
            x_tiles[n] = ts

        def xs(n, kc):
            return x_tiles[n][kc // 2][:, kc % 2, :]

        def qk_group(n, mc, wsb, dst):
            p = pA.tile([128, QC], f32, tag="pA")
            for kc in range(8):
                nc.tensor.matmul(
                    out=p[:], lhsT=wsb[:, kc, mc * 128:(mc + 1) * 128],
                    rhs=xs(n, kc), start=(kc == 0), stop=(kc == 7))
            nc.vector.tensor_copy(dst[:, mc, n * QC:(n + 1) * QC], p[:])

        def v_group(n, mt):
            p = pA.tile([128, QC], f32, tag="pA")
            for kc in range(8):
                nc.tensor.matmul(
                    out=p[:], lhsT=xs(n, kc)[:, mt * 128:(mt + 1) * 128],
                    rhs=wv_sb[:, kc, :], start=(kc == 0), stop=(kc == 7))
            gm = n * 4 + mt
            nc.vector.tensor_copy(
                vp[:, gm, :, 0:D],
                p.rearrange("p (h d) -> p h d", d=D))

        def p3_group(qc, j, n2):
            mt = 4 * qc + j
            p = pA.tile([128, QC], f32, tag="pA")
            for kd in range(4):
                nc.tensor.matmul(
                    out=p[:],
                    lhsT=ot[:, kd, mt * 128:(mt + 1) * 128],
                    rhs=wc_sb[:, kd, n2 * QC:(n2 + 1) * QC],
                    start=(kd == 0), stop=(kd == 3))
            st = stp.tile([128, QC], bf16, tag="st")
            nc.vector.tensor_copy(st[:], p[:])
            nc.sync.dma_start(
                out=out[mt * 128:(mt + 1) * 128, n2 * QC:(n2 + 1) * QC],
                in_=st[:])

        def so_block(qc):
            K = 4 * qc + 4
            LAG = 3
            for hp in range(4):
                ha, hb = 2 * hp, 2 * hp + 1
                po_a = pop.tile([128, QC], f32, tag="po")
                po_b = pop.tile([128, QC], f32, tag="po")
                ews = {}

                def o_pair(kc, qc=qc, hp=hp, po_a=po_a, po_b=po_b, ews=ews):
                    off = (kc - 4 * qc) * 128 if kc >= 4 * qc else 0
                    ew = ews.pop(kc)
                    for hi, (h, po_t) in ((0, (ha, po_a)), (1, (hb, po_b))):
                        nc.tensor.matmul(
                            out=po_t[0:D + 1, off:QC],
                            lhsT=vp[:, kc, h, :],
                            rhs=ew[:, hi, off:QC],
                            start=(kc == 0), stop=(kc == K - 1),
                            skip_group_check=True)

                for kc in range(K):
                    off = (kc - 4 * qc) * 128 if kc >= 4 * qc else 0
                    pw_t = pwp.tile([128, 2, QC], f32, tag="pw")
                    for hi, r0 in ((0, 0), (1, 64)):
                        nc.tensor.matmul(
                            out=pw_t[:, hi, off:QC],
                            lhsT=kt[r0:r0 + 64, hp,
                                    kc * 128:(kc + 1) * 128],
                            rhs=qt[r0:r0 + 64, hp,
                                   qc * QC + off:(qc + 1) * QC],
                            start=True, stop=True, tile_position=(r0, 0))
                    ew = ewp.tile([128, 2, QC], bf16, tag="ew")
                    ews[kc] = ew
                    nc.scalar.activation(
                        ew[:, :, off:QC], pw_t[:, :, off:QC], Exp,
                        scale=0.125)
                    if kc >= 4 * qc:
                        nc.gpsimd.tensor_mul(
                            ew[:, :, off:off + 128],
                            ew[:, :, off:off + 128],
                            tril_sb[:, :, :])
                    if kc >= LAG:
                        o_pair(kc - LAG)
                    yield
                for kc in range(max(0, K - LAG), K):
                    o_pair(kc)
                qsl = slice(qc * QC, (qc + 1) * QC)
                nc.vector.tensor_copy(ot[0:64, hp, qsl], po_a[0:D, :])
                nc.vector.tensor_copy(ot[64:128, hp, qsl], po_b[0:D, :])
                d_sb = dbp.tile([1, 2, QC], f32, tag="dsb")
                nc.vector.tensor_copy(d_sb[0:1, 0, :], po_a[D:D + 1, :])
                nc.vector.tensor_copy(d_sb[0:1, 1, :], po_b[D:D + 1, :])
                dr = drp.tile([2, QC], f32, tag="dr")
                nc.sync.dma_start(out=dr[:], in_=d_sb[:])
                db = dbp.tile([128, QC], f32, tag="db")
                d0 = dr[:]
                nc.gpsimd.dma_start(
                    out=db[:],
                    in_=bass.AP(tensor=d0.tensor, offset=d0.offset,
                                ap=[[QC, 2], [0, 64], [1, QC]]))
                nc.vector.reciprocal_approx_fast(db[:], db[:])
                nc.vector.tensor_mul(ot[:, hp, qsl], ot[:, hp, qsl], db[:])

        def block_fillers(n):
            fs = []
            if n + 1 < NQC:
                fs.append(lambda n=n: emit_x_dma(n + 1))
            if n < NQC:
                for mc in range(4):
                    fs.append(lambda n=n, mc=mc: qk_group(n, mc, wq_sb, qt))
                    fs.append(lambda n=n, mc=mc: qk_group(n, mc, wk_sb, kt))
                for mt in range(4):
                    fs.append(lambda n=n, mt=mt: v_group(n, mt))
            if n >= 4:
                for qc in (2 * (n - 4), 2 * (n - 4) + 1):
                    for j in range(4):
                        for n2 in range(2):
                            fs.append(
                                lambda qc=qc, j=j, n2=n2: p3_group(qc, j, n2))
            return fs

        emit_x_dma(0)
        for n in range(6):
            fillers = block_fillers(n)
            if n == 0 or n == 5:
                for f in fillers:
                    f()
                continue
            qc = n - 1
            n_bi = 4 * (4 * qc + 4)
            rate = len(fillers) / n_bi
            acc = 0.0
            for _ in so_block(qc):
                acc += rate
                while acc >= 1.0 and fillers:
                    fillers.pop(0)()
                    acc -= 1.0
            for f in fillers:
                f()


def build_program():
    if "nc" in _CACHE:
        return _CACHE["nc"]
    import concourse.bacc as bacc
    import concourse.tile as tile
    from concourse import mybir

    f32, bf16 = mybir.dt.float32, mybir.dt.bfloat16
    nc = bacc.Bacc("TRN2", target_bir_lowering=False, debug=False,
                   num_devices=N_CORES)
    io = {
        "xT": nc.dram_tensor("xT", [C, T], bf16, kind="ExternalInput").ap(),
        "wq": nc.dram_tensor("wq", [C, HL], bf16, kind="ExternalInput").ap(),
        "wk": nc.dram_tensor("wk", [C, HL], bf16, kind="ExternalInput").ap(),
        "wv": nc.dram_tensor("wv", [C, HL], bf16, kind="ExternalInput").ap(),
        "wc": nc.dram_tensor("wc", [HL, C], bf16, kind="ExternalInput").ap(),
        "tril": nc.dram_tensor("tril", [128, 128], bf16,
                               kind="ExternalInput").ap(),
        "out": nc.dram_tensor("out", [T, C], bf16, kind="ExternalOutput").ap(),
    }
    with tile.TileContext(nc) as tc:
        _emit(nc, tc, tile, mybir, io)
    nc.compile()
    _CACHE["nc"] = nc
    return nc


def make_in_maps(x, Wq, Wk, Wv, Wc):
    bf16 = ml_dtypes.bfloat16
    x = np.asarray(x, dtype=np.float32)
    Wq = np.asarray(Wq, dtype=np.float32).astype(bf16)
    Wk = np.asarray(Wk, dtype=np.float32).astype(bf16)
    Wv = np.asarray(Wv, dtype=np.float32).astype(bf16)
    Wc = np.asarray(Wc, dtype=np.float32).astype(bf16)

    i_idx = np.arange(128)[:, None]
    j_idx = np.arange(128)[None, :]
    tril = (j_idx >= i_idx).astype(bf16)

    in_maps = []
    for b in range(B):
        xT = np.ascontiguousarray(x[b].T).astype(bf16)
        for g in range(2):
            sl = slice(g * HL, (g + 1) * HL)
            in_maps.append({
                "xT": xT,
                "wq": np.ascontiguousarray(Wq[:, sl]),
                "wk": np.ascontiguousarray(Wk[:, sl]),
                "wv": np.ascontiguousarray(Wv[:, sl]),
                "wc": np.ascontiguousarray(Wc[sl, :]),
                "tril": tril,
            })
    return in_maps


def kernel(x, Wq, Wk, Wv, Wc, bc):
    from concourse.bass_utils import run_bass_kernel_spmd

    nc = build_program()
    in_maps = make_in_maps(x, Wq, Wk, Wv, Wc)
    res = run_bass_kernel_spmd(nc, in_maps, core_ids=list(range(N_CORES)))
    bc = np.asarray(bc, dtype=np.float32)
    out = np.empty((B, T, C), dtype=np.float32)
    for b in range(B):
        out[b] = (res.results[2 * b]["out"].astype(np.float32)
                  + res.results[2 * b + 1]["out"].astype(np.float32) + bc)
    return out
